# Initial kernel scaffold
#
"""Trainium2 Bass kernel for StyleGAN2-style upsampled Conv1d.

Reference computation (for x:(16,256,4096), weight:(256,256,3), bias:(256,)):
  y = conv_transpose1d(x, weight, stride=2)      # correlation on 2x-dilated x
  z = upfirdn1d(y, [1,3,3,1]/8 * 2)              # depthwise FIR
  out = z + bias                                  # (16, 256, 8192)

The transposed conv + FIR collapse into TWO 3-tap correlations over the
original x grid (even/odd output phases):
  out[:, :, 2j]   = A @x[j-1] + B @x[j]  + C @x[j+1]
  out[:, :, 2j+1] = A'@x[j-1] + B'@x[j]  + C'@x[j+1]
with (w0,w1,w2) = weight taps:
  A  = .75 w0 + .25 w1   B  = .25 w0 + .75 w1 + .75 w2   C  = .25 w2
  A' = .25 w0            B' = .75 w0 + .75 w1 + .25 w2   C' = .25 w1 + .75 w2

On-chip: chunk-major streaming — for each NCHUNK-position chunk, 12
accumulating fp32r matmuls (2 phases x 3 taps x 2 K-tiles) fill one
PSUM pair, which drains immediately (bias add + even/odd interleave in
one vector or scalar op) and DMAs out.  Draining each pair right after
its matmuls keeps the PSUM pool from ever stalling the PE.  Sharding:
data-parallel over batch (2 per core x 8 cores).

DMA model (measured): every queue has ~2.5us issue-to-data latency;
SWDGE sustains ~390 GB/s, each HWDGE queue ~190 GB/s, all sharing
~400-450 GB/s of HBM.  So the critical first tiles (x chunk-0 heads +
m=0 weights, ~1.3 MB) are split across all three queues in parallel
and everything else queues strictly behind them.
"""

import numpy as np

import concourse.bass as bass
import concourse.mybir as mybir
import concourse.tile as tile
from concourse import bacc
from concourse.bass_utils import run_bass_kernel_spmd

N, IN_CH, OUT_CH, KERNEL, D = 16, 256, 256, 3, 4096
NCORES = 8
BPC = N // NCORES          # batches per core
DOUT = 2 * D
F32 = mybir.dt.float32
F32R = mybir.dt.float32r

NCHUNK = 512               # matmul moving free dim (= one PSUM bank of fp32)
NCHUNKS = D // NCHUNK
HEAD = NCHUNK + 3          # x columns needed by chunk 0
NWARM = 10                 # leading dummy matmuls: clock ramp while DMAs land
NTAIL = 8                  # trailing dummies: hold the clock through epilogue

_CACHED = {}


def _wblk(phase, tap, k, m):
    # m-major so each m-half of the weights is one contiguous DMA
    return ((m * 2 + phase) * 3 + tap) * 2 + k


def _build_nc(mm_dtype=F32R):
    nc = bacc.Bacc("TRN2", target_bir_lowering=False, debug=False)

    # x arrives host-padded with zero columns at 0 and D+1 (3-tap halo).
    # float32r is bit-identical to float32; declaring DRAM sides as f32r
    # lets the HWDGE queues (sync/scalar) carry them without a cast.
    x_t = nc.dram_tensor("x", [BPC, IN_CH, D + 2], mm_dtype, kind="ExternalInput")
    # w layout: 24 blocks of (128 K, 128 M); see _wblk
    w_t = nc.dram_tensor("w", [128, 24 * 128], mm_dtype, kind="ExternalInput")
    b_t = nc.dram_tensor("b", [128, 2], F32, kind="ExternalInput")
    o_t = nc.dram_tensor("out", [BPC, OUT_CH, DOUT], F32, kind="ExternalOutput")

    pbufs = 8 * 512 // (2 * NCHUNK)   # PSUM pairs that fit in 8 banks
    with tile.TileContext(nc) as tc:
        with (
            tc.tile_pool(name="wpool", bufs=1) as wpool,
            tc.tile_pool(name="xpool", bufs=2 * BPC) as xpool,
            tc.tile_pool(name="zpool", bufs=2 * pbufs) as zpool,
            tc.tile_pool(name="ppool", bufs=pbufs, space="PSUM") as ppool,
        ):
            w_sb = wpool.tile([128, 24 * 128], mm_dtype)
            b_sb = wpool.tile([128, 2], F32)
            x_sb = {}
            for bb in range(BPC):
                for k in range(2):
                    x_sb[bb, k] = xpool.tile(
                        [128, D + 2], mm_dtype, tag="x", name=f"x_{bb}_{k}"
                    )

            # Measured: early DMA bandwidth is ONE shared ~400 GB/s pool
            # with a ~2us issue-to-data lag — splitting across queues
            # does not add bandwidth, it only lets non-critical bytes
            # steal from critical ones.  So every input rides the single
            # SWDGE stream in exact first-use order; HWDGE queues carry
            # only the 1KB bias and, later, the output stores.
            nc.scalar.dma_start(out=b_sb[:], in_=b_t[:])
            for k in range(2):
                nc.gpsimd.dma_start(
                    out=x_sb[0, k][:, 0:HEAD],
                    in_=x_t[0, k * 128:(k + 1) * 128, 0:HEAD],
                )
            nc.gpsimd.dma_start(out=w_sb[:, 0:768], in_=w_t[:, 0:768])
            nc.gpsimd.dma_start(out=w_sb[:, 768:1536], in_=w_t[:, 768:1536])
            cuts = list(range(HEAD, D + 2, 515)) + [D + 2]
            for t in range(len(cuts) - 1):
                for k in range(2):
                    lo, hi = cuts[t], cuts[t + 1]
                    nc.gpsimd.dma_start(
                        out=x_sb[0, k][:, lo:hi],
                        in_=x_t[0, k * 128:(k + 1) * 128, lo:hi],
                    )
            nc.gpsimd.dma_start(out=w_sb[:, 1536:3072], in_=w_t[:, 1536:3072])
            for k in range(2):
                nc.gpsimd.dma_start(
                    out=x_sb[1, k][:], in_=x_t[1, k * 128:(k + 1) * 128, :]
                )

            # PE warmup while the first DMAs land: dummy bf16 matmuls on
            # a memset tile run the HAM clock-gate ramp (~3us + margin)
            # so the real stream starts at the full 2.4 GHz.  The PSUM
            # garbage lands in a pool slot that a later real
            # accumulation group's start=True clears.
            warm_bf = wpool.tile([128, 128 + 512], mybir.dt.bfloat16)
            nc.vector.memset(warm_bf[:], 1.0)
            warm_ps = ppool.tile([128, 2 * NCHUNK], F32, tag="pair", name="warm_ps")
            for _ in range(NWARM):
                nc.tensor.matmul(
                    warm_ps[:, 0:512],
                    lhsT=warm_bf[:, 0:128],
                    rhs=warm_bf[:, 128:640],
                    start=True,
                    stop=True,
                )

            seq = [(bb, m, c) for bb in range(BPC) for m in range(2)
                   for c in range(NCHUNKS)]
            for ci, (bb, m, c) in enumerate(seq):
                pair = ppool.tile([128, 2 * NCHUNK], F32, tag="pair",
                                  name=f"pair_{bb}_{m}_{c}")
                for phase in range(2):
                    for tap in range(3):
                        for k in range(2):
                            w_ap = w_sb[:, _wblk(phase, tap, k, m) * 128:][:, :128]
                            rhs = x_sb[bb, k][:, NCHUNK * c + tap:NCHUNK * c + tap + NCHUNK]
                            nc.tensor.matmul(
                                pair[:, phase * NCHUNK:(phase + 1) * NCHUNK],
                                lhsT=w_ap,
                                rhs=rhs,
                                start=(tap == 0 and k == 0),
                                stop=(tap == 2 and k == 1),
                            )
                bias_ap = b_sb[:, m:m + 1]
                zt = zpool.tile([128, 2 * NCHUNK], F32, tag="z",
                                name=f"z_{bb}_{m}_{c}")
                # psum pair is [even(NCHUNK) | odd(NCHUNK)]; writing in
                # (phase, j) order at stride 2 interleaves the two
                # phases while adding bias.
                last = ci == len(seq) - 1
                vin = pair[:].rearrange("p (two j) -> p two j", two=2)
                if not last:
                    vout = zt[:].rearrange("p (j two) -> p two j", two=2)
                    if ci % 2 == 0:
                        nc.vector.tensor_scalar(
                            out=vout, in0=vin,
                            scalar1=bias_ap, scalar2=None,
                            op0=mybir.AluOpType.add,
                        )
                    else:
                        nc.scalar.activation(
                            out=vout, in_=vin,
                            func=mybir.ActivationFunctionType.Identity,
                            bias=bias_ap,
                        )
                    oeng = nc.sync if ci % 2 == 0 else nc.scalar
                    oeng.dma_start(
                        out=o_t[bb, m * 128:(m + 1) * 128,
                                c * 2 * NCHUNK:(c + 1) * 2 * NCHUNK],
                        in_=zt[:],
                    )
                else:
                    # Final chunk: split the drain across vector+scalar
                    # and the store across both HWDGE queues so the
                    # kernel tail is half as long.
                    for h, dma_eng in enumerate((nc.sync, nc.scalar)):
                        vout = zt[:, h * NCHUNK:(h + 1) * NCHUNK].rearrange(
                            "p (j two) -> p two j", two=2
                        )
                        vin_h = vin[:, :, h * (NCHUNK // 2):(h + 1) * (NCHUNK // 2)]
                        if h == 0:
                            nc.vector.tensor_scalar(
                                out=vout, in0=vin_h,
                                scalar1=bias_ap, scalar2=None,
                                op0=mybir.AluOpType.add,
                            )
                        else:
                            nc.scalar.activation(
                                out=vout, in_=vin_h,
                                func=mybir.ActivationFunctionType.Identity,
                                bias=bias_ap,
                            )
                        dma_eng.dma_start(
                            out=o_t[bb, m * 128:(m + 1) * 128,
                                    c * 2 * NCHUNK + h * NCHUNK:
                                    c * 2 * NCHUNK + (h + 1) * NCHUNK],
                            in_=zt[:, h * NCHUNK:(h + 1) * NCHUNK],
                        )

            # Trailing dummies keep the PE active (and the clock gate at
            # 8/8) while the final drains + stores retire, so the
            # framework's epilogue barrier doesn't run at 1/2 clock.
            tail_ps = ppool.tile([128, 2 * NCHUNK], F32, tag="pair",
                                 name="tail_ps")
            for _ in range(NTAIL):
                nc.tensor.matmul(
                    tail_ps[:, 0:512],
                    lhsT=warm_bf[:, 0:128],
                    rhs=warm_bf[:, 128:640],
                    start=True,
                    stop=True,
                )
    nc.compile()
    return nc


def _host_weights(weight, bias):
    w = np.asarray(weight, dtype=np.float32)
    w0, w1, w2 = w[:, :, 0], w[:, :, 1], w[:, :, 2]
    taps = [
        [0.75 * w0 + 0.25 * w1, 0.25 * w0 + 0.75 * w1 + 0.75 * w2, 0.25 * w2],
        [0.25 * w0, 0.75 * w0 + 0.75 * w1 + 0.25 * w2, 0.25 * w1 + 0.75 * w2],
    ]
    w_host = np.zeros((128, 24 * 128), dtype=np.float32)
    for phase in range(2):
        for tap in range(3):
            for k in range(2):
                for m in range(2):
                    blk = _wblk(phase, tap, k, m)
                    # lhsT block[i, o] = W[phase][tap][m*128+o, k*128+i]
                    wt = taps[phase][tap][m * 128:(m + 1) * 128, k * 128:(k + 1) * 128]
                    w_host[:, blk * 128:(blk + 1) * 128] = wt.T
    b_host = np.asarray(bias, dtype=np.float32).reshape(2, 128).T.copy()
    return w_host, b_host


def _host_x(x):
    x = np.asarray(x, dtype=np.float32)
    return np.ascontiguousarray(np.pad(x, ((0, 0), (0, 0), (1, 1))))


def kernel(x, weight, bias):
    x = _host_x(x)
    w_host, b_host = _host_weights(weight, bias)

    if "nc" not in _CACHED:
        _CACHED["nc"] = _build_nc()
    nc = _CACHED["nc"]

    in_maps = []
    for core in range(NCORES):
        shard = np.ascontiguousarray(x[core * BPC:(core + 1) * BPC])
        in_maps.append({"x": shard, "w": w_host, "b": b_host})

    res = run_bass_kernel_spmd(nc, in_maps, core_ids=list(range(NCORES)))
    out = np.concatenate([np.asarray(r["out"]) for r in res.results], axis=0)
    return out



# revision 1
# speedup vs baseline: 2.1282x; 2.1282x over previous
"""Trainium2 Bass kernel for StyleGAN2-style upsampled Conv1d.

Reference computation (for x:(16,256,4096), weight:(256,256,3), bias:(256,)):
  y = conv_transpose1d(x, weight, stride=2)      # correlation on 2x-dilated x
  z = upfirdn1d(y, [1,3,3,1]/8 * 2)              # depthwise FIR
  out = z + bias                                  # (16, 256, 8192)

The transposed conv + FIR collapse into TWO 3-tap correlations over the
original x grid (even/odd output phases):
  out[:, :, 2j]   = A @x[j-1] + B @x[j]  + C @x[j+1]
  out[:, :, 2j+1] = A'@x[j-1] + B'@x[j]  + C'@x[j+1]
with (w0,w1,w2) = weight taps:
  A  = .75 w0 + .25 w1   B  = .25 w0 + .75 w1 + .75 w2   C  = .25 w2
  A' = .25 w0            B' = .75 w0 + .75 w1 + .25 w2   C' = .25 w1 + .75 w2

On-chip: chunk-major streaming — for each NCHUNK-position chunk, 12
accumulating fp32r matmuls (2 phases x 3 taps x 2 K-tiles) fill one
PSUM pair, which drains immediately (bias add + even/odd interleave in
one vector or scalar op) and DMAs out.  Draining each pair right after
its matmuls keeps the PSUM pool from ever stalling the PE.  Sharding:
data-parallel over batch (2 per core x 8 cores).

DMA model (measured): every queue has ~2.5us issue-to-data latency;
SWDGE sustains ~390 GB/s, each HWDGE queue ~190 GB/s, all sharing
~400-450 GB/s of HBM.  So the critical first tiles (x chunk-0 heads +
m=0 weights, ~1.3 MB) are split across all three queues in parallel
and everything else queues strictly behind them.
"""

import numpy as np

import concourse.bass as bass
import concourse.mybir as mybir
import concourse.tile as tile
from concourse import bacc
from concourse.bass_utils import run_bass_kernel_spmd

N, IN_CH, OUT_CH, KERNEL, D = 16, 256, 256, 3, 4096
NCORES = 8
BPC = N // NCORES          # batches per core
DOUT = 2 * D
F32 = mybir.dt.float32
F32R = mybir.dt.float32r

NCHUNK = 512               # matmul moving free dim (= one PSUM bank of fp32)
NCHUNKS = D // NCHUNK
HEAD = NCHUNK + 3          # x columns needed by chunk 0
NWARM = 10                 # leading dummy matmuls: clock ramp while DMAs land
NTAIL = 8                  # trailing dummies: hold the clock through epilogue

_CACHED = {}


def _wblk(phase, tap, k, m):
    # m-major so each m-half of the weights is one contiguous DMA
    return ((m * 2 + phase) * 3 + tap) * 2 + k


def _build_nc(mm_dtype=F32R):
    nc = bacc.Bacc("TRN2", target_bir_lowering=False, debug=False)

    # x arrives host-padded with zero columns at 0 and D+1 (3-tap halo).
    # float32r is bit-identical to float32; declaring DRAM sides as f32r
    # lets the HWDGE queues (sync/scalar) carry them without a cast.
    x_t = nc.dram_tensor("x", [BPC, IN_CH, D + 2], mm_dtype, kind="ExternalInput")
    # w layout: 24 blocks of (128 K, 128 M); see _wblk
    w_t = nc.dram_tensor("w", [128, 24 * 128], mm_dtype, kind="ExternalInput")
    b_t = nc.dram_tensor("b", [128, 2], F32, kind="ExternalInput")
    o_t = nc.dram_tensor("out", [BPC, OUT_CH, DOUT], F32, kind="ExternalOutput")

    pbufs = 8 * 512 // (2 * NCHUNK)   # PSUM pairs that fit in 8 banks
    with tile.TileContext(nc) as tc:
        with (
            tc.tile_pool(name="wpool", bufs=1) as wpool,
            tc.tile_pool(name="xpool", bufs=2 * BPC) as xpool,
            tc.tile_pool(name="zpool", bufs=2 * pbufs) as zpool,
            tc.tile_pool(name="ppool", bufs=pbufs, space="PSUM") as ppool,
        ):
            w_sb = wpool.tile([128, 24 * 128], mm_dtype)
            b_sb = wpool.tile([128, 2], F32)
            x_sb = {}
            for bb in range(BPC):
                for k in range(2):
                    x_sb[bb, k] = xpool.tile(
                        [128, D + 2], mm_dtype, tag="x", name=f"x_{bb}_{k}"
                    )

            # Measured: early DMA bandwidth is ONE shared ~400 GB/s pool
            # with a ~2us issue-to-data lag — splitting across queues
            # does not add bandwidth, it only lets non-critical bytes
            # steal from critical ones.  So every input rides the single
            # SWDGE stream in exact first-use order; HWDGE queues carry
            # only the 1KB bias and, later, the output stores.
            nc.scalar.dma_start(out=b_sb[:], in_=b_t[:])
            for k in range(2):
                nc.gpsimd.dma_start(
                    out=x_sb[0, k][:, 0:HEAD],
                    in_=x_t[0, k * 128:(k + 1) * 128, 0:HEAD],
                )
            nc.gpsimd.dma_start(out=w_sb[:, 0:768], in_=w_t[:, 0:768])
            nc.gpsimd.dma_start(out=w_sb[:, 768:1536], in_=w_t[:, 768:1536])
            cuts = list(range(HEAD, D + 2, 515)) + [D + 2]
            for t in range(len(cuts) - 1):
                for k in range(2):
                    lo, hi = cuts[t], cuts[t + 1]
                    nc.gpsimd.dma_start(
                        out=x_sb[0, k][:, lo:hi],
                        in_=x_t[0, k * 128:(k + 1) * 128, lo:hi],
                    )
            nc.gpsimd.dma_start(out=w_sb[:, 1536:3072], in_=w_t[:, 1536:3072])
            for k in range(2):
                nc.gpsimd.dma_start(
                    out=x_sb[1, k][:], in_=x_t[1, k * 128:(k + 1) * 128, :]
                )

            # PE warmup while the first DMAs land: dummy bf16 matmuls on
            # a memset tile run the HAM clock-gate ramp (~3us + margin)
            # so the real stream starts at the full 2.4 GHz.  The PSUM
            # garbage lands in a pool slot that a later real
            # accumulation group's start=True clears.
            warm_bf = wpool.tile([128, 128 + 512], mybir.dt.bfloat16)
            nc.vector.memset(warm_bf[:], 1.0)
            warm_ps = ppool.tile([128, 2 * NCHUNK], F32, tag="pair", name="warm_ps")
            for _ in range(NWARM):
                nc.tensor.matmul(
                    warm_ps[:, 0:512],
                    lhsT=warm_bf[:, 0:128],
                    rhs=warm_bf[:, 128:640],
                    start=True,
                    stop=True,
                )

            seq = [(bb, m, c) for bb in range(BPC) for m in range(2)
                   for c in range(NCHUNKS)]
            for ci, (bb, m, c) in enumerate(seq):
                pair = ppool.tile([128, 2 * NCHUNK], F32, tag="pair",
                                  name=f"pair_{bb}_{m}_{c}")
                for phase in range(2):
                    for tap in range(3):
                        for k in range(2):
                            w_ap = w_sb[:, _wblk(phase, tap, k, m) * 128:][:, :128]
                            rhs = x_sb[bb, k][:, NCHUNK * c + tap:NCHUNK * c + tap + NCHUNK]
                            nc.tensor.matmul(
                                pair[:, phase * NCHUNK:(phase + 1) * NCHUNK],
                                lhsT=w_ap,
                                rhs=rhs,
                                start=(tap == 0 and k == 0),
                                stop=(tap == 2 and k == 1),
                            )
                bias_ap = b_sb[:, m:m + 1]
                zt = zpool.tile([128, 2 * NCHUNK], F32, tag="z",
                                name=f"z_{bb}_{m}_{c}")
                # psum pair is [even(NCHUNK) | odd(NCHUNK)]; writing in
                # (phase, j) order at stride 2 interleaves the two
                # phases while adding bias.
                last = ci == len(seq) - 1
                vin = pair[:].rearrange("p (two j) -> p two j", two=2)
                if not last:
                    vout = zt[:].rearrange("p (j two) -> p two j", two=2)
                    if ci % 2 == 0:
                        nc.vector.tensor_scalar(
                            out=vout, in0=vin,
                            scalar1=bias_ap, scalar2=None,
                            op0=mybir.AluOpType.add,
                        )
                    else:
                        nc.scalar.activation(
                            out=vout, in_=vin,
                            func=mybir.ActivationFunctionType.Identity,
                            bias=bias_ap,
                        )
                    oeng = nc.sync if ci % 2 == 0 else nc.scalar
                    oeng.dma_start(
                        out=o_t[bb, m * 128:(m + 1) * 128,
                                c * 2 * NCHUNK:(c + 1) * 2 * NCHUNK],
                        in_=zt[:],
                    )
                else:
                    # Final chunk: split the drain across vector+scalar
                    # and the store across both HWDGE queues so the
                    # kernel tail is half as long.
                    for h, dma_eng in enumerate((nc.sync, nc.scalar)):
                        vout = zt[:, h * NCHUNK:(h + 1) * NCHUNK].rearrange(
                            "p (j two) -> p two j", two=2
                        )
                        vin_h = vin[:, :, h * (NCHUNK // 2):(h + 1) * (NCHUNK // 2)]
                        if h == 0:
                            nc.vector.tensor_scalar(
                                out=vout, in0=vin_h,
                                scalar1=bias_ap, scalar2=None,
                                op0=mybir.AluOpType.add,
                            )
                        else:
                            nc.scalar.activation(
                                out=vout, in_=vin_h,
                                func=mybir.ActivationFunctionType.Identity,
                                bias=bias_ap,
                            )
                        dma_eng.dma_start(
                            out=o_t[bb, m * 128:(m + 1) * 128,
                                    c * 2 * NCHUNK + h * NCHUNK:
                                    c * 2 * NCHUNK + (h + 1) * NCHUNK],
                            in_=zt[:, h * NCHUNK:(h + 1) * NCHUNK],
                        )

            # Trailing dummies keep the PE active (and the clock gate at
            # 8/8) while the final drains + stores retire, so the
            # framework's epilogue barrier doesn't run at 1/2 clock.
            tail_ps = ppool.tile([128, 2 * NCHUNK], F32, tag="pair",
                                 name="tail_ps")
            for _ in range(NTAIL):
                nc.tensor.matmul(
                    tail_ps[:, 0:512],
                    lhsT=warm_bf[:, 0:128],
                    rhs=warm_bf[:, 128:640],
                    start=True,
                    stop=True,
                )
    nc.compile()
    return nc


def _host_weights(weight, bias):
    w = np.asarray(weight, dtype=np.float32)
    w0, w1, w2 = w[:, :, 0], w[:, :, 1], w[:, :, 2]
    taps = [
        [0.75 * w0 + 0.25 * w1, 0.25 * w0 + 0.75 * w1 + 0.75 * w2, 0.25 * w2],
        [0.25 * w0, 0.75 * w0 + 0.75 * w1 + 0.25 * w2, 0.25 * w1 + 0.75 * w2],
    ]
    w_host = np.zeros((128, 24 * 128), dtype=np.float32)
    for phase in range(2):
        for tap in range(3):
            for k in range(2):
                for m in range(2):
                    blk = _wblk(phase, tap, k, m)
                    # lhsT block[i, o] = W[phase][tap][m*128+o, k*128+i]
                    wt = taps[phase][tap][m * 128:(m + 1) * 128, k * 128:(k + 1) * 128]
                    w_host[:, blk * 128:(blk + 1) * 128] = wt.T
    b_host = np.asarray(bias, dtype=np.float32).reshape(2, 128).T.copy()
    return w_host, b_host


def _host_x(x):
    x = np.asarray(x, dtype=np.float32)
    return np.ascontiguousarray(np.pad(x, ((0, 0), (0, 0), (1, 1))))


def kernel(x, weight, bias):
    x = _host_x(x)
    w_host, b_host = _host_weights(weight, bias)

    if "nc" not in _CACHED:
        _CACHED["nc"] = _build_nc()
    nc = _CACHED["nc"]

    in_maps = []
    for core in range(NCORES):
        shard = np.ascontiguousarray(x[core * BPC:(core + 1) * BPC])
        in_maps.append({"x": shard, "w": w_host, "b": b_host})

    res = run_bass_kernel_spmd(nc, in_maps, core_ids=list(range(NCORES)))
    out = np.concatenate([np.asarray(r["out"]) for r in res.results], axis=0)
    return out



# revision 4
# speedup vs baseline: 2.2701x; 1.0667x over previous
"""Trainium2 Bass kernel for StyleGAN2-style upsampled Conv1d.

Reference computation (for x:(16,256,4096), weight:(256,256,3), bias:(256,)):
  y = conv_transpose1d(x, weight, stride=2)      # correlation on 2x-dilated x
  z = upfirdn1d(y, [1,3,3,1]/8 * 2)              # depthwise FIR
  out = z + bias                                  # (16, 256, 8192)

The transposed conv + FIR collapse into TWO 3-tap correlations over the
original x grid (even/odd output phases):
  out[:, :, 2j]   = A @x[j-1] + B @x[j]  + C @x[j+1]
  out[:, :, 2j+1] = A'@x[j-1] + B'@x[j]  + C'@x[j+1]
with (w0,w1,w2) = weight taps:
  A  = .75 w0 + .25 w1   B  = .25 w0 + .75 w1 + .75 w2   C  = .25 w2
  A' = .25 w0            B' = .75 w0 + .75 w1 + .25 w2   C' = .25 w1 + .75 w2

On-chip: chunk-major streaming — for each NCHUNK-position chunk, 12
accumulating fp32r matmuls (2 phases x 3 taps x 2 K-tiles) fill one
PSUM pair, which drains immediately (bias add + even/odd interleave in
one vector or scalar op) and DMAs out.  Draining each pair right after
its matmuls keeps the PSUM pool from ever stalling the PE.  Sharding:
data-parallel over batch (2 per core x 8 cores).

DMA model (measured): every queue has ~2.5us issue-to-data latency;
SWDGE sustains ~390 GB/s, each HWDGE queue ~190 GB/s, all sharing
~400-450 GB/s of HBM.  So the critical first tiles (x chunk-0 heads +
m=0 weights, ~1.3 MB) are split across all three queues in parallel
and everything else queues strictly behind them.
"""

import numpy as np

import concourse.bass as bass
import concourse.mybir as mybir
import concourse.tile as tile
from concourse import bacc
from concourse.bass_utils import run_bass_kernel_spmd

N, IN_CH, OUT_CH, KERNEL, D = 16, 256, 256, 3, 4096
NCORES = 8
BPC = N // NCORES          # batches per core
DOUT = 2 * D
F32 = mybir.dt.float32
F32R = mybir.dt.float32r

BF16 = mybir.dt.bfloat16

NCHUNK = 512               # matmul moving free dim (= one PSUM bank of fp32)
NCHUNKS = D // NCHUNK
HEAD = NCHUNK + 3          # x columns needed by chunk 0
NWARM = 10                 # leading dummy matmuls: clock ramp while DMAs land
NTAIL = 8                  # trailing dummies: hold the clock through epilogue

_CACHED = {}


def _wblk(phase, tap, k, m):
    # m-major so each m-half of the weights is one contiguous DMA
    return ((m * 2 + phase) * 3 + tap) * 2 + k


def _build_nc(mm_dtype=BF16):
    nc = bacc.Bacc("TRN2", target_bir_lowering=False, debug=False)

    # x arrives host-padded with zero columns at 0 and D+1 (3-tap halo),
    # pre-cast to bf16 (tolerance is 2e-2; bf16 rounding costs ~4e-3).
    # bf16 halves every DMA byte, halves LDWEIGHTS/SBUF traffic, and the
    # PE still accumulates in fp32 PSUM at the same 1 row/cycle.
    x_t = nc.dram_tensor("x", [BPC, IN_CH, D + 2], mm_dtype, kind="ExternalInput")
    # w layout: 24 blocks of (128 K, 128 M); see _wblk
    w_t = nc.dram_tensor("w", [128, 24 * 128], mm_dtype, kind="ExternalInput")
    b_t = nc.dram_tensor("b", [128, 2], F32, kind="ExternalInput")
    # Output leaves the chip as bf16 (half the store bytes); the host
    # upcasts back to fp32.
    o_t = nc.dram_tensor("out", [BPC, OUT_CH, DOUT], BF16, kind="ExternalOutput")

    pbufs = 8 * 512 // (2 * NCHUNK)   # PSUM pairs that fit in 8 banks
    with tile.TileContext(nc) as tc:
        with (
            tc.tile_pool(name="wpool", bufs=1) as wpool,
            tc.tile_pool(name="xpool", bufs=2 * BPC) as xpool,
            tc.tile_pool(name="zpool", bufs=2 * pbufs) as zpool,
            tc.tile_pool(name="ppool", bufs=pbufs, space="PSUM") as ppool,
        ):
            w_sb = wpool.tile([128, 24 * 128], mm_dtype)
            b_sb = wpool.tile([128, 2], F32)
            x_sb = {}
            for bb in range(BPC):
                for k in range(2):
                    x_sb[bb, k] = xpool.tile(
                        [128, D + 2], mm_dtype, tag="x", name=f"x_{bb}_{k}"
                    )

            # Measured: early DMA bandwidth is ONE shared ~400 GB/s pool
            # with a ~2us issue-to-data lag — splitting across queues
            # does not add bandwidth, it only lets non-critical bytes
            # steal from critical ones.  So every input rides the single
            # SWDGE stream in exact first-use order; HWDGE queues carry
            # only the 1KB bias and, later, the output stores.
            nc.scalar.dma_start(out=b_sb[:], in_=b_t[:])
            for k in range(2):
                nc.gpsimd.dma_start(
                    out=x_sb[0, k][:, 0:HEAD],
                    in_=x_t[0, k * 128:(k + 1) * 128, 0:HEAD],
                )
            nc.gpsimd.dma_start(out=w_sb[:, 0:768], in_=w_t[:, 0:768])
            nc.gpsimd.dma_start(out=w_sb[:, 768:1536], in_=w_t[:, 768:1536])
            cuts = list(range(HEAD, D + 2, 515)) + [D + 2]
            for t in range(len(cuts) - 1):
                for k in range(2):
                    lo, hi = cuts[t], cuts[t + 1]
                    nc.gpsimd.dma_start(
                        out=x_sb[0, k][:, lo:hi],
                        in_=x_t[0, k * 128:(k + 1) * 128, lo:hi],
                    )
            nc.gpsimd.dma_start(out=w_sb[:, 1536:3072], in_=w_t[:, 1536:3072])
            for k in range(2):
                nc.gpsimd.dma_start(
                    out=x_sb[1, k][:], in_=x_t[1, k * 128:(k + 1) * 128, :]
                )

            # PE warmup while the first DMAs land: dummy bf16 matmuls on
            # a memset tile run the HAM clock-gate ramp (~3us + margin)
            # so the real stream starts at the full 2.4 GHz.  The PSUM
            # garbage lands in a pool slot that a later real
            # accumulation group's start=True clears.
            warm_bf = wpool.tile([128, 128 + 512], mybir.dt.bfloat16)
            nc.vector.memset(warm_bf[:], 1.0)
            warm_ps = ppool.tile([128, 2 * NCHUNK], F32, tag="pair", name="warm_ps")
            for _ in range(NWARM):
                nc.tensor.matmul(
                    warm_ps[:, 0:512],
                    lhsT=warm_bf[:, 0:128],
                    rhs=warm_bf[:, 128:640],
                    start=True,
                    stop=True,
                )

            seq = [(bb, m, c) for bb in range(BPC) for m in range(2)
                   for c in range(NCHUNKS)]
            for ci, (bb, m, c) in enumerate(seq):
                pair = ppool.tile([128, 2 * NCHUNK], F32, tag="pair",
                                  name=f"pair_{bb}_{m}_{c}")
                for phase in range(2):
                    for tap in range(3):
                        for k in range(2):
                            w_ap = w_sb[:, _wblk(phase, tap, k, m) * 128:][:, :128]
                            rhs = x_sb[bb, k][:, NCHUNK * c + tap:NCHUNK * c + tap + NCHUNK]
                            nc.tensor.matmul(
                                pair[:, phase * NCHUNK:(phase + 1) * NCHUNK],
                                lhsT=w_ap,
                                rhs=rhs,
                                start=(tap == 0 and k == 0),
                                stop=(tap == 2 and k == 1),
                            )
                bias_ap = b_sb[:, m:m + 1]
                zt = zpool.tile([128, 2 * NCHUNK], BF16, tag="z",
                                name=f"z_{bb}_{m}_{c}")
                # psum pair is [even(NCHUNK) | odd(NCHUNK)]; writing in
                # (phase, j) order at stride 2 interleaves the two
                # phases while adding bias.
                last = ci == len(seq) - 1
                vin = pair[:].rearrange("p (two j) -> p two j", two=2)
                if not last:
                    vout = zt[:].rearrange("p (j two) -> p two j", two=2)
                    if ci % 2 == 0:
                        nc.vector.tensor_scalar(
                            out=vout, in0=vin,
                            scalar1=bias_ap, scalar2=None,
                            op0=mybir.AluOpType.add,
                        )
                    else:
                        nc.scalar.activation(
                            out=vout, in_=vin,
                            func=mybir.ActivationFunctionType.Identity,
                            bias=bias_ap,
                        )
                    oeng = nc.sync if ci % 2 == 0 else nc.scalar
                    oeng.dma_start(
                        out=o_t[bb, m * 128:(m + 1) * 128,
                                c * 2 * NCHUNK:(c + 1) * 2 * NCHUNK],
                        in_=zt[:],
                    )
                else:
                    # Final chunk: split the drain across vector+scalar
                    # and the store across both HWDGE queues so the
                    # kernel tail is half as long.
                    for h, dma_eng in enumerate((nc.sync, nc.scalar)):
                        vout = zt[:, h * NCHUNK:(h + 1) * NCHUNK].rearrange(
                            "p (j two) -> p two j", two=2
                        )
                        vin_h = vin[:, :, h * (NCHUNK // 2):(h + 1) * (NCHUNK // 2)]
                        if h == 0:
                            nc.vector.tensor_scalar(
                                out=vout, in0=vin_h,
                                scalar1=bias_ap, scalar2=None,
                                op0=mybir.AluOpType.add,
                            )
                        else:
                            nc.scalar.activation(
                                out=vout, in_=vin_h,
                                func=mybir.ActivationFunctionType.Identity,
                                bias=bias_ap,
                            )
                        dma_eng.dma_start(
                            out=o_t[bb, m * 128:(m + 1) * 128,
                                    c * 2 * NCHUNK + h * NCHUNK:
                                    c * 2 * NCHUNK + (h + 1) * NCHUNK],
                            in_=zt[:, h * NCHUNK:(h + 1) * NCHUNK],
                        )

            # Trailing dummies keep the PE active (and the clock gate at
            # 8/8) while the final drains + stores retire, so the
            # framework's epilogue barrier doesn't run at 1/2 clock.
            tail_ps = ppool.tile([128, 2 * NCHUNK], F32, tag="pair",
                                 name="tail_ps")
            for _ in range(NTAIL):
                nc.tensor.matmul(
                    tail_ps[:, 0:512],
                    lhsT=warm_bf[:, 0:128],
                    rhs=warm_bf[:, 128:640],
                    start=True,
                    stop=True,
                )
    nc.compile()
    return nc


def _host_weights(weight, bias):
    w = np.asarray(weight, dtype=np.float32)
    w0, w1, w2 = w[:, :, 0], w[:, :, 1], w[:, :, 2]
    taps = [
        [0.75 * w0 + 0.25 * w1, 0.25 * w0 + 0.75 * w1 + 0.75 * w2, 0.25 * w2],
        [0.25 * w0, 0.75 * w0 + 0.75 * w1 + 0.25 * w2, 0.25 * w1 + 0.75 * w2],
    ]
    w_host = np.zeros((128, 24 * 128), dtype=np.float32)
    for phase in range(2):
        for tap in range(3):
            for k in range(2):
                for m in range(2):
                    blk = _wblk(phase, tap, k, m)
                    # lhsT block[i, o] = W[phase][tap][m*128+o, k*128+i]
                    wt = taps[phase][tap][m * 128:(m + 1) * 128, k * 128:(k + 1) * 128]
                    w_host[:, blk * 128:(blk + 1) * 128] = wt.T
    b_host = np.asarray(bias, dtype=np.float32).reshape(2, 128).T.copy()
    import ml_dtypes
    return w_host.astype(ml_dtypes.bfloat16), b_host


def _host_x(x):
    import ml_dtypes
    x = np.asarray(x, dtype=np.float32)
    xp = np.pad(x, ((0, 0), (0, 0), (1, 1)))
    return np.ascontiguousarray(xp.astype(ml_dtypes.bfloat16))


def kernel(x, weight, bias):
    x = _host_x(x)
    w_host, b_host = _host_weights(weight, bias)

    if "nc" not in _CACHED:
        _CACHED["nc"] = _build_nc()
    nc = _CACHED["nc"]

    in_maps = []
    for core in range(NCORES):
        shard = np.ascontiguousarray(x[core * BPC:(core + 1) * BPC])
        in_maps.append({"x": shard, "w": w_host, "b": b_host})

    res = run_bass_kernel_spmd(nc, in_maps, core_ids=list(range(NCORES)))
    out = np.concatenate(
        [np.asarray(r["out"]).astype(np.float32) for r in res.results], axis=0
    )
    return out

